# revision 17
# baseline (speedup 1.0000x reference)
"""Trainium2 Bass kernel: DCT frequency encoder (masked DCT -> conv1 -> conv2 -> pool -> proj).

Self-contained: hardcodes shapes B=32, C=1, H=W=224, mid=128, emb=256, 8 cores,
4 images per core (pure data parallelism over the batch).

Algorithmic structure (validated against the reference to ~4e-6 rel err):
  - The reference masks the 2-D DCT to the top-left 112x112 block, so only
    B112 @ x @ B112^T is computed (B112 = first 112 rows of the DCT-II basis).
  - BatchNorms are folded into conv weights/biases on the host.
  - Outside the 114x114 top-left window, conv2's input is a per-channel
    constant, so relu(bn2(conv2)) there is a closed-form constant field that
    is summed on the host and folded into the output bias. The device only
    computes the varying 114x114 window per image and its spatial sum.
"""

import numpy as np

_NCORES = 8
_IPC = 4           # images per core
_N = 224
_KC = 112          # kept DCT coeffs per axis
_CW = 115          # conv1 output window size (h1 rows/cols 0..114)
_VW = 114          # conv2 output window size
_HPW = 116         # padded Hp width (h1 coords -1..114)
_CH = 116          # conv1 output rows computed (must give even matmul N chunks)
_NPIX = float(_N * _N)

_PROG = {}


def _dct_basis112():
    k = np.arange(_N, dtype=np.float64)[:, None]
    n = np.arange(_N, dtype=np.float64)[None, :]
    B = 2.0 * np.cos(np.pi * k * (2.0 * n + 1.0) / (2.0 * _N))
    return B[:_KC]  # (112, 224)


def _prep_consts(w1, b1, g1, be1, m1, v1, w2, b2, g2, be2, m2, v2, wp, bp):
    f = np.float64
    w1, b1, g1, be1, m1, v1 = [np.asarray(a, f) for a in (w1, b1, g1, be1, m1, v1)]
    w2, b2, g2, be2, m2, v2 = [np.asarray(a, f) for a in (w2, b2, g2, be2, m2, v2)]
    wp, bp = np.asarray(wp, f), np.asarray(bp, f)

    s1 = g1 / np.sqrt(v1 + 1e-5)
    W1 = w1[:, 0] * s1[:, None, None]              # (64,3,3)
    B1 = b1 * s1 + (be1 - m1 * s1)                 # (64,)
    s2 = g2 / np.sqrt(v2 + 1e-5)
    W2 = w2 * s2[:, None, None, None]              # (128,64,3,3)
    B2 = b2 * s2 + (be2 - m2 * s2)                 # (128,)
    c1 = np.maximum(B1, 0.0)

    # Constant-region pooled sum: positions outside the varying 114x114 window.
    # A[oc,i,j] = sum_{dh,dw} q[oc,dh,dw]*u_dh(i)*u_dw(j) with u encoding SAME
    # padding tap validity; separable, so only 9 (row-class, col-class) cases.
    q = np.einsum('oikl,i->okl', W2, c1)           # (128,3,3)
    u = np.ones((3, _N)); u[0, 0] = 0.0; u[2, _N - 1] = 0.0
    # row/col classes: 0 -> border-left/top (idx 0), 1 -> middle, 2 -> border-right/bottom
    cls = np.ones(_N, np.int64); cls[0] = 0; cls[_N - 1] = 2
    uc = np.stack([u[:, 0], u[:, 1], u[:, _N - 1]], axis=1)   # (3 taps, 3 classes)
    maskvar = np.zeros((_N, _N), bool); maskvar[:_VW, :_VW] = True
    counts = np.zeros((3, 3))
    ii, jj = np.meshgrid(cls, cls, indexing='ij')
    for a in range(3):
        for bcl in range(3):
            counts[a, bcl] = np.sum((ii == a) & (jj == bcl) & ~maskvar)
    S_const = np.zeros(128)
    for a in range(3):
        for bcl in range(3):
            if counts[a, bcl] == 0:
                continue
            A = np.einsum('okl,k,l->o', q, uc[:, a], uc[:, bcl])
            S_const += counts[a, bcl] * np.maximum(A + B2, 0.0)

    # Transposed-spatial convention: the device computes G = X2^T, so swap
    # each 3x3 tap's (dh, dw).
    W1p = W1.transpose(0, 2, 1)
    W2p = W2.transpose(0, 1, 3, 2)

    Bb = _dct_basis112()                            # (112, 224)
    bt = np.ascontiguousarray(Bb.T)                 # (224, 112)

    w1im = np.zeros((9, 64))
    for dh in range(3):
        for dw in range(3):
            w1im[3 * dh + dw] = W1p[:, dh, dw]
    b1rep = np.concatenate([B1, B1])[:, None]       # (128,1)

    w2pair = np.zeros((3, 128, 128))
    w2sing = np.zeros((3, 64, 128))
    for w in range(3):
        w2pair[w, :64] = W2p[:, :, 0, w].T          # [ic, oc]
        w2pair[w, 64:] = W2p[:, :, 1, w].T
        w2sing[w] = W2p[:, :, 2, w].T
    b2t = B2[:, None]                               # (128,1)

    wps = wp.T / _NPIX                              # (128,256)
    brow = (bp + (S_const @ wp.T) / _NPIX)[None, :]  # (1,256)

    c = lambda a: np.ascontiguousarray(a, np.float32)
    return {
        'btE': c(bt[0::2]), 'btO': c(bt[1::2]),
        'zflat': c(np.zeros((2, 7376))),
        'bt2a': c(bt[0:112]), 'bt2b': c(bt[112:224]),
        'ident': c(np.eye(_KC)),
        'w1im': c(w1im), 'b1rep': c(b1rep),
        'w2pair': c(w2pair), 'w2sing': c(w2sing), 'b2t': c(b2t),
        'wps': c(wps), 'ones1': c(np.ones((1, _IPC))), 'brow': c(brow),
        'zeros': c(np.zeros((128, 512))),
    }


def _build_program():
    import sys
    if '/opt/trn_rl_repo' not in sys.path:
        sys.path.insert(0, '/opt/trn_rl_repo')
    from contextlib import ExitStack
    import concourse.bass as bass
    import concourse.tile as tile
    from concourse import bacc, mybir

    f32 = mybir.dt.float32
    f32r = mybir.dt.float32r
    AF = mybir.ActivationFunctionType
    ALU = mybir.AluOpType
    AX = mybir.AxisListType

    nc = bacc.Bacc("TRN2", target_bir_lowering=False, debug=False,
                   num_devices=_NCORES)

    x_ap = nc.dram_tensor("x", [_IPC, _N, _N], f32r, kind="ExternalInput").ap()
    din = lambda name, shape, dt=f32r: nc.dram_tensor(name, shape, dt, kind="ExternalInput").ap()
    d_btE = din("btE", [112, _KC]); d_btO = din("btO", [112, _KC])
    d_zflat = din("zflat", [2, 7376])
    d_bt2a = din("bt2a", [112, _KC]); d_bt2b = din("bt2b", [112, _KC])
    d_ident = din("ident", [_KC, _KC])
    d_w1im = din("w1im", [9, 64]); d_b1rep = din("b1rep", [128, 1], f32)
    d_w2pair = din("w2pair", [3, 128, 128]); d_w2sing = din("w2sing", [3, 64, 128])
    d_b2t = din("b2t", [128, 1], f32)
    d_wps = din("wps", [128, 256]); d_ones1 = din("ones1", [1, _IPC])
    d_brow = din("brow", [1, 256])
    d_zeros = din("zeros", [128, 512])
    out_ap = nc.dram_tensor("out", [_IPC, 256], f32, kind="ExternalOutput").ap()

    with tile.TileContext(nc) as tc, ExitStack() as ctx:
        cpool = ctx.enter_context(tc.tile_pool(name="const", bufs=1))

        def cload(dram_ap, shape, dt=f32r, tag=None):
            t = cpool.tile(shape, dt, tag=tag or dram_ap.tensor.name, name=tag or dram_ap.tensor.name)
            nc.sync.dma_start(t[:], dram_ap)
            return t

        c_btE = cload(d_btE, [112, _KC])
        c_btO = cload(d_btO, [112, _KC])
        c_bt2a = cload(d_bt2a, [112, _KC])
        c_bt2b = cload(d_bt2b, [112, _KC])
        c_ident = cload(d_ident, [_KC, _KC])
        c_w1im = cload(d_w1im, [9, 64])
        c_b1rep = cload(d_b1rep, [128, 1], f32)
        c_w2p = [cload(d_w2pair[w], [128, 128], tag=f"w2p{w}") for w in range(3)]
        c_w2s = [cload(d_w2sing[w], [64, 128], tag=f"w2s{w}") for w in range(3)]
        c_b2t = cload(d_b2t, [128, 1], f32)
        c_wps = cload(d_wps, [128, 256])
        c_ones1 = cload(d_ones1, [1, _IPC])
        c_brow = cload(d_brow, [1, 256])
        c_zeros = cload(d_zeros, [128, 512], tag="zeros")

        misc = ctx.enter_context(tc.tile_pool(name="misc", bufs=1))
        xpool = ctx.enter_context(tc.tile_pool(name="x", bufs=2))
        t1pool = ctx.enter_context(tc.tile_pool(name="t1", bufs=2))

        t1t0 = misc.tile([112, 448], f32r, tag="t1t0", name="t1t0")
        t1t1 = misc.tile([112, 448], f32r, tag="t1t1", name="t1t1")
        stats = misc.tile([128, 32 * _IPC], f32, tag="stats", name="stats")
        means = misc.tile([128, _IPC], f32r, tag="means", name="means")

        # ---------------- DCT phase ----------------
        # flatX: padded DCT image, flattened row-major with 117-wide rows
        # (cols 112..116 zero) split across 2 partitions with overlap so conv1
        # im2col reads are single contiguous runs.
        #   partition 0: [pad, X rows 0..57]   partition 1: [pad, X rows 54..116]
        with tc.tile_pool(name="dctps", bufs=2, space="PSUM") as dctps, \
             tc.tile_pool(name="trps", bufs=2, space="PSUM") as trpsp:
            for img in range(_IPC):
                x2r = xpool.tile([112, 448], f32r, tag="x2r", name="x2r")
                nc.gpsimd.dma_start(
                    x2r[:], x_ap[img].rearrange("(p two) w -> p (two w)", two=2))
                t1ps = dctps.tile([112, 224], f32, tag="t1ps", name="t1ps")
                nc.tensor.matmul(t1ps[:], c_btE[:], x2r[:, 0:224], start=True, stop=False)
                nc.tensor.matmul(t1ps[:], c_btO[:], x2r[:, 224:448], start=False, stop=True)
                t1s = t1pool.tile([112, 224], f32r, tag="t1s", name="t1s")
                nc.scalar.copy(t1s[:], t1ps[:])
                for chn in range(2):
                    trp = trpsp.tile([112, 112], f32r, tag="trp", name="trp")
                    nc.tensor.transpose(
                        trp[:], t1s[:, 112 * chn:112 * chn + 112], c_ident[:])
                    dstt = (t1t0, t1t1)[chn]
                    nc.vector.tensor_copy(dstt[:, 112 * img:112 * img + 112], trp[:])
            gps = dctps.tile([112, 448], f32, tag="gps", name="gps", bufs=1)
            nc.tensor.matmul(gps[:], c_bt2a[:], t1t0[:], start=True, stop=False)
            nc.tensor.matmul(gps[:], c_bt2b[:], t1t1[:], start=False, stop=True)
            gsb = misc.tile([112, 448], f32r, tag="gsb", name="gsb")
            nc.scalar.copy(gsb[:], gps[:])
        # ---------------- conv phase ----------------
        i1pool = ctx.enter_context(tc.tile_pool(name="i1", bufs=2))
        fxpool = ctx.enter_context(tc.tile_pool(name="fx", bufs=1))
        vppool = ctx.enter_context(tc.tile_pool(name="vp", bufs=2))
        scpool = ctx.enter_context(tc.tile_pool(name="scr", bufs=2))
        c1psp = ctx.enter_context(tc.tile_pool(name="c1ps", bufs=3, space="PSUM"))
        c2psp = ctx.enter_context(tc.tile_pool(name="c2ps", bufs=3, space="PSUM"))

        # (r0, nrows, flat_partition): seg covers conv1 output rows [r0, r0+nrows)
        SEGS = [(0, 28, 0), (28, 28, 0), (56, 28, 1), (84, 32, 1)]
        _SW = 117  # seg row pitch (115 valid cols + 2 wrap-junk cols)
        vps = [None] * _IPC

        def conv1(img):
            vp = vppool.tile([128, 117 * _HPW], f32r, tag="vp", name="vp")
            vps[img] = vp
            vpv = vp[:].rearrange("p (r c) -> p r c", c=_HPW)
            # zero strips: lo Hp row 0 (top image pad), col 0 (left pad) in both halves
            nc.sync.dma_start(vpv[0:64, 0:1, 0:_HPW], c_zeros[0:64, 0:_HPW])
            nc.sync.dma_start(vpv[0:128, 0:117, 0:1], c_zeros[0:128, 0:117])
            fx = fxpool.tile([2, 7376], f32r, tag="fx", name="fx")
            fxv0 = fx[0:1, 1:7372].rearrange("p (r c) -> p r c", c=117)
            fxv1 = fx[1:2, 1:7372].rearrange("p (r c) -> p r c", c=117)
            nc.gpsimd.dma_start(fx[:], d_zflat)
            g = gsb[:, 112 * img:112 * img + 112]
            nc.gpsimd.dma_start(fxv0[0:1, 0:58, 0:112], g[0:58, :])
            nc.gpsimd.dma_start(fxv1[0:1, 0:58, 0:112], g[54:112, :])
            for (r0, nrows, fp) in SEGS:
                seg = i1pool.tile([9, 32 * _SW], f32r, tag="i1seg", name="i1seg")
                base = 1 - fp * (54 * 117)  # flat offset of X row 0 col 0 in this partition
                for dh in range(3):
                    for dw in range(3):
                        part = 3 * dh + dw
                        if r0 == 0 and dh == 0:
                            nc.sync.dma_start(seg[part:part + 1, 0:_SW],
                                              c_zeros[part:part + 1, 0:_SW])
                            nc.sync.dma_start(
                                seg[part:part + 1, _SW:nrows * _SW],
                                fx[fp:fp + 1, base + dw - 1:
                                   base + dw - 1 + (nrows - 1) * _SW])
                        else:
                            o = base + (r0 + dh - 1) * _SW + dw - 1
                            nc.sync.dma_start(seg[part:part + 1, 0:nrows * _SW],
                                              fx[fp:fp + 1, o:o + nrows * _SW])
                done = 0
                while done < nrows:
                    cr = min(4, nrows - done)
                    r = r0 + done
                    ps1 = c1psp.tile([64, cr * _SW], f32, tag="c1ps", name="c1ps")
                    rhs = seg[:, done * _SW:(done + cr) * _SW]
                    nc.tensor.matmul(ps1[:], c_w1im[:], rhs, start=True, stop=True)
                    ps1v = ps1[:].rearrange("p (r c) -> p r c", c=_SW)
                    # lo half: Hp rows r+1..r+cr (ACT relu+bias), skip 2 junk cols
                    nc.scalar.activation(vpv[0:64, r + 1:r + 1 + cr, 1:_HPW],
                                         ps1v[:, 0:cr, 0:115], AF.Relu,
                                         bias=c_b1rep[0:64, :])
                    # hi half (partition-shifted write): Hp+116 -> hi rows r.. (DVE)
                    nc.vector.tensor_scalar(vpv[64:128, r:r + cr, 1:_HPW],
                                            ps1v[:, 0:cr, 0:115], c_b1rep[0:64, :],
                                            0.0, op0=ALU.add, op1=ALU.max)
                    done += cr

        CH2 = [(r2, 4) for r2 in range(0, 108, 4)] + [(108, 3), (111, 3)]

        def conv2(img):
            vp = vps[img]
            vpv = vp[:].rearrange("p (r c) -> p r c", c=_HPW)
            for ci, (r2, cr) in enumerate(CH2):
                ps2 = c2psp.tile([128, cr * _VW], f32, tag="c2ps", name="c2ps")
                for w in range(3):
                    nc.tensor.matmul(ps2[:], c_w2p[w][:],
                                     vpv[0:128, r2:r2 + cr, w:w + _VW],
                                     start=(w == 0), stop=False)
                for w in range(3):
                    nc.tensor.matmul(ps2[:], c_w2s[w][:],
                                     vpv[0:64, r2 + 2:r2 + 2 + cr, w:w + _VW],
                                     start=False, stop=(w == 2))
                scr = scpool.tile([128, 4 * _VW], f32, tag="scr", name="scr")
                nc.scalar.activation(scr[:, 0:cr * _VW], ps2[:], AF.Relu,
                                     bias=c_b2t[:],
                                     accum_out=stats[:, 32 * img + ci:32 * img + ci + 1])

        conv1(0)
        conv1(1)
        conv2(0)
        conv1(2)
        conv2(1)
        conv1(3)
        conv2(2)
        conv2(3)

        with nc.allow_low_precision(reason="float32r holds identical fp32 bits"):
            for img in range(_IPC):
                nc.vector.reduce_sum(means[:, img:img + 1],
                                     stats[:, 32 * img:32 * img + 29], axis=AX.X)
        psf = c2psp.tile([_IPC, 256], f32, tag="psf", name="psf", bufs=1)
        nc.tensor.matmul(psf[:], means[:], c_wps[:], start=True, stop=False)
        nc.tensor.matmul(psf[:], c_ones1[:], c_brow[:], start=False, stop=True)
        outs = misc.tile([_IPC, 256], f32, tag="outs", name="outs")
        nc.scalar.copy(outs[:], psf[:])
        nc.sync.dma_start(out_ap[:], outs[:])

    nc.compile()
    return nc


_last_results = None


def kernel(x, w1, b1, g1, be1, m1, v1, w2, b2, g2, be2, m2, v2, wp, bp):
    import sys
    if '/opt/trn_rl_repo' not in sys.path:
        sys.path.insert(0, '/opt/trn_rl_repo')
    from concourse.bass_utils import run_bass_kernel_spmd

    global _last_results
    if 'nc' not in _PROG:
        _PROG['nc'] = _build_program()
    nc = _PROG['nc']

    consts = _prep_consts(w1, b1, g1, be1, m1, v1, w2, b2, g2, be2, m2, v2, wp, bp)
    x = np.ascontiguousarray(np.asarray(x, np.float32).reshape(_B_TOTAL, _N, _N))
    in_maps = []
    for c in range(_NCORES):
        m = dict(consts)
        m['x'] = x[c * _IPC:(c + 1) * _IPC]
        in_maps.append(m)
    res = run_bass_kernel_spmd(nc, in_maps, core_ids=list(range(_NCORES)))
    _last_results = res
    out = np.concatenate([res.results[c]['out'] for c in range(_NCORES)], axis=0)
    return out.astype(np.float32)


_B_TOTAL = 32


# revision 18
# speedup vs baseline: 1.5544x; 1.5544x over previous
"""Trainium2 Bass kernel: DCT frequency encoder (masked DCT -> conv1 -> conv2 -> pool -> proj).

Self-contained: hardcodes shapes B=32, C=1, H=W=224, mid=128, emb=256, 8 cores,
4 images per core (pure data parallelism over the batch).

Algorithmic structure (validated against the reference to ~4e-6 rel err):
  - The reference masks the 2-D DCT to the top-left 112x112 block, so only
    B112 @ x @ B112^T is computed (B112 = first 112 rows of the DCT-II basis).
  - BatchNorms are folded into conv weights/biases on the host.
  - Outside the 114x114 top-left window, conv2's input is a per-channel
    constant, so relu(bn2(conv2)) there is a closed-form constant field that
    is summed on the host and folded into the output bias. The device only
    computes the varying 114x114 window per image and its spatial sum.
"""

import numpy as np

_NCORES = 8
_IPC = 4           # images per core
_N = 224
_KC = 112          # kept DCT coeffs per axis
_CW = 115          # conv1 output window size (h1 rows/cols 0..114)
_VW = 114          # conv2 output window size
_HPW = 116         # padded Hp width (h1 coords -1..114)
_CH = 116          # conv1 output rows computed (must give even matmul N chunks)
_NPIX = float(_N * _N)

_PROG = {}


def _dct_basis112():
    k = np.arange(_N, dtype=np.float64)[:, None]
    n = np.arange(_N, dtype=np.float64)[None, :]
    B = 2.0 * np.cos(np.pi * k * (2.0 * n + 1.0) / (2.0 * _N))
    return B[:_KC]  # (112, 224)


def _prep_consts(w1, b1, g1, be1, m1, v1, w2, b2, g2, be2, m2, v2, wp, bp):
    f = np.float64
    w1, b1, g1, be1, m1, v1 = [np.asarray(a, f) for a in (w1, b1, g1, be1, m1, v1)]
    w2, b2, g2, be2, m2, v2 = [np.asarray(a, f) for a in (w2, b2, g2, be2, m2, v2)]
    wp, bp = np.asarray(wp, f), np.asarray(bp, f)

    s1 = g1 / np.sqrt(v1 + 1e-5)
    W1 = w1[:, 0] * s1[:, None, None]              # (64,3,3)
    B1 = b1 * s1 + (be1 - m1 * s1)                 # (64,)
    s2 = g2 / np.sqrt(v2 + 1e-5)
    W2 = w2 * s2[:, None, None, None]              # (128,64,3,3)
    B2 = b2 * s2 + (be2 - m2 * s2)                 # (128,)
    c1 = np.maximum(B1, 0.0)

    # Constant-region pooled sum: positions outside the varying 114x114 window.
    # A[oc,i,j] = sum_{dh,dw} q[oc,dh,dw]*u_dh(i)*u_dw(j) with u encoding SAME
    # padding tap validity; separable, so only 9 (row-class, col-class) cases.
    q = np.einsum('oikl,i->okl', W2, c1)           # (128,3,3)
    u = np.ones((3, _N)); u[0, 0] = 0.0; u[2, _N - 1] = 0.0
    # row/col classes: 0 -> border-left/top (idx 0), 1 -> middle, 2 -> border-right/bottom
    cls = np.ones(_N, np.int64); cls[0] = 0; cls[_N - 1] = 2
    uc = np.stack([u[:, 0], u[:, 1], u[:, _N - 1]], axis=1)   # (3 taps, 3 classes)
    maskvar = np.zeros((_N, _N), bool); maskvar[:_VW, :_VW] = True
    counts = np.zeros((3, 3))
    ii, jj = np.meshgrid(cls, cls, indexing='ij')
    for a in range(3):
        for bcl in range(3):
            counts[a, bcl] = np.sum((ii == a) & (jj == bcl) & ~maskvar)
    S_const = np.zeros(128)
    for a in range(3):
        for bcl in range(3):
            if counts[a, bcl] == 0:
                continue
            A = np.einsum('okl,k,l->o', q, uc[:, a], uc[:, bcl])
            S_const += counts[a, bcl] * np.maximum(A + B2, 0.0)

    # Transposed-spatial convention: the device computes G = X2^T, so swap
    # each 3x3 tap's (dh, dw).
    W1p = W1.transpose(0, 2, 1)
    W2p = W2.transpose(0, 1, 3, 2)

    Bb = _dct_basis112()                            # (112, 224)
    bt = np.ascontiguousarray(Bb.T)                 # (224, 112)

    w1im = np.zeros((112, 64))
    for dh in range(3):
        for dw in range(3):
            w1im[3 * dh + dw] = W1p[:, dh, dw]
    b1rep = np.concatenate([B1, B1])[:, None]       # (128,1)

    w2pair = np.zeros((3, 128, 128))
    w2sing = np.zeros((3, 128, 128))    # hi half zero: K=128 matmul, rows 64+ contribute 0
    for w in range(3):
        w2pair[w, :64] = W2p[:, :, 0, w].T          # [ic, oc]
        w2pair[w, 64:] = W2p[:, :, 1, w].T
        w2sing[w, :64] = W2p[:, :, 2, w].T
    b2t = B2[:, None]                               # (128,1)

    wps = wp.T / _NPIX                              # (128,256)
    brow = (bp + (S_const @ wp.T) / _NPIX)[None, :]  # (1,256)

    c = lambda a: np.ascontiguousarray(a, np.float32)
    return {
        'btE': c(bt[0::2]), 'btO': c(bt[1::2]),
        'zflat': c(np.zeros((2, 7376))),
        'bt2a': c(bt[0:112]), 'bt2b': c(bt[112:224]),
        'ident': c(np.eye(_KC)),
        'w1im': c(w1im), 'b1rep': c(b1rep),
        'w2pair': c(w2pair), 'w2sing': c(w2sing), 'b2t': c(b2t),
        'wps': c(wps), 'ones1': c(np.ones((1, _IPC))), 'brow': c(brow),
        'zeros': c(np.zeros((128, 512))),
        'zseg': c(np.zeros((103, 3744))),
    }


def _build_program():
    import sys
    if '/opt/trn_rl_repo' not in sys.path:
        sys.path.insert(0, '/opt/trn_rl_repo')
    from contextlib import ExitStack
    import concourse.bass as bass
    import concourse.tile as tile
    from concourse import bacc, mybir

    f32 = mybir.dt.float32
    f32r = mybir.dt.float32r
    AF = mybir.ActivationFunctionType
    ALU = mybir.AluOpType
    AX = mybir.AxisListType

    nc = bacc.Bacc("TRN2", target_bir_lowering=False, debug=False,
                   num_devices=_NCORES)

    x_ap = nc.dram_tensor("x", [_IPC, _N, _N], f32r, kind="ExternalInput").ap()
    din = lambda name, shape, dt=f32r: nc.dram_tensor(name, shape, dt, kind="ExternalInput").ap()
    d_btE = din("btE", [112, _KC]); d_btO = din("btO", [112, _KC])
    d_zflat = din("zflat", [2, 7376])
    d_bt2a = din("bt2a", [112, _KC]); d_bt2b = din("bt2b", [112, _KC])
    d_ident = din("ident", [_KC, _KC])
    d_w1im = din("w1im", [112, 64]); d_b1rep = din("b1rep", [128, 1], f32)
    d_w2pair = din("w2pair", [3, 128, 128]); d_w2sing = din("w2sing", [3, 128, 128])
    d_zseg = din("zseg", [103, 3744])
    d_b2t = din("b2t", [128, 1], f32)
    d_wps = din("wps", [128, 256]); d_ones1 = din("ones1", [1, _IPC])
    d_brow = din("brow", [1, 256])
    d_zeros = din("zeros", [128, 512])
    out_ap = nc.dram_tensor("out", [_IPC, 256], f32, kind="ExternalOutput").ap()

    with tile.TileContext(nc) as tc, ExitStack() as ctx:
        cpool = ctx.enter_context(tc.tile_pool(name="const", bufs=1))

        def cload(dram_ap, shape, dt=f32r, tag=None):
            t = cpool.tile(shape, dt, tag=tag or dram_ap.tensor.name, name=tag or dram_ap.tensor.name)
            nc.sync.dma_start(t[:], dram_ap)
            return t

        c_btE = cload(d_btE, [112, _KC])
        c_btO = cload(d_btO, [112, _KC])
        c_bt2a = cload(d_bt2a, [112, _KC])
        c_bt2b = cload(d_bt2b, [112, _KC])
        c_ident = cload(d_ident, [_KC, _KC])
        c_w1im = cload(d_w1im, [112, 64])
        c_b1rep = cload(d_b1rep, [128, 1], f32)
        c_w2p = [cload(d_w2pair[w], [128, 128], tag=f"w2p{w}") for w in range(3)]
        c_w2s = [cload(d_w2sing[w], [128, 128], tag=f"w2s{w}") for w in range(3)]
        c_b2t = cload(d_b2t, [128, 1], f32)
        c_wps = cload(d_wps, [128, 256])
        c_ones1 = cload(d_ones1, [1, _IPC])
        c_brow = cload(d_brow, [1, 256])
        c_zeros = cload(d_zeros, [128, 512], tag="zeros")

        misc = ctx.enter_context(tc.tile_pool(name="misc", bufs=1))
        xpool = ctx.enter_context(tc.tile_pool(name="x", bufs=2))
        t1pool = ctx.enter_context(tc.tile_pool(name="t1", bufs=2))

        t1t0 = misc.tile([112, 448], f32r, tag="t1t0", name="t1t0")
        segts = [misc.tile([112, 3744], f32r, tag=f"seg{i}", name=f"seg{i}")
                 for i in range(2)]
        for s in segts:
            nc.scalar.dma_start(s[9:112, :], d_zseg)
        t1t1 = misc.tile([112, 448], f32r, tag="t1t1", name="t1t1")
        stats = misc.tile([128, 32 * _IPC], f32, tag="stats", name="stats")
        means = misc.tile([128, _IPC], f32r, tag="means", name="means")

        # ---------------- DCT phase ----------------
        # flatX: padded DCT image, flattened row-major with 117-wide rows
        # (cols 112..116 zero) split across 2 partitions with overlap so conv1
        # im2col reads are single contiguous runs.
        #   partition 0: [pad, X rows 0..57]   partition 1: [pad, X rows 54..116]
        with tc.tile_pool(name="dctps", bufs=2, space="PSUM") as dctps, \
             tc.tile_pool(name="trps", bufs=2, space="PSUM") as trpsp:
            for img in range(_IPC):
                x2r = xpool.tile([112, 448], f32r, tag="x2r", name="x2r")
                nc.scalar.dma_start(
                    x2r[:], x_ap[img].rearrange("(p two) w -> p (two w)", two=2))
                t1ps = dctps.tile([112, 224], f32, tag="t1ps", name="t1ps")
                nc.tensor.matmul(t1ps[:], c_btE[:], x2r[:, 0:224], start=True, stop=False)
                nc.tensor.matmul(t1ps[:], c_btO[:], x2r[:, 224:448], start=False, stop=True)
                t1s = t1pool.tile([112, 224], f32r, tag="t1s", name="t1s")
                nc.scalar.copy(t1s[:], t1ps[:])
                for chn in range(2):
                    trp = trpsp.tile([112, 112], f32r, tag="trp", name="trp")
                    nc.tensor.transpose(
                        trp[:], t1s[:, 112 * chn:112 * chn + 112], c_ident[:])
                    dstt = (t1t0, t1t1)[chn]
                    nc.vector.tensor_copy(dstt[:, 112 * img:112 * img + 112], trp[:])
            gps = dctps.tile([112, 448], f32, tag="gps", name="gps", bufs=1)
            nc.tensor.matmul(gps[:], c_bt2a[:], t1t0[:], start=True, stop=False)
            nc.tensor.matmul(gps[:], c_bt2b[:], t1t1[:], start=False, stop=True)
            gsb = misc.tile([112, 448], f32r, tag="gsb", name="gsb")
            nc.scalar.copy(gsb[:], gps[:])
        # ---------------- conv phase ----------------
        i1pool = ctx.enter_context(tc.tile_pool(name="i1", bufs=2))
        fxpool = ctx.enter_context(tc.tile_pool(name="fx", bufs=1))
        vppool = ctx.enter_context(tc.tile_pool(name="vp", bufs=2))
        scpool = ctx.enter_context(tc.tile_pool(name="scr", bufs=2))
        c1psp = ctx.enter_context(tc.tile_pool(name="c1ps", bufs=3, space="PSUM"))
        c2psp = ctx.enter_context(tc.tile_pool(name="c2ps", bufs=3, space="PSUM"))

        # (r0, nrows, flat_partition): seg covers conv1 output rows [r0, r0+nrows)
        SEGS = [(0, 28, 0), (28, 28, 0), (56, 28, 1), (84, 32, 1)]
        _SW = 117  # seg row pitch (115 valid cols + 2 wrap-junk cols)
        vps = [None] * _IPC

        def conv1(img):
            vp = vppool.tile([128, 117 * _HPW], f32r, tag="vp", name="vp")
            vps[img] = vp
            vpv = vp[:].rearrange("p (r c) -> p r c", c=_HPW)
            # zero strips: lo Hp row 0 (top image pad), col 0 (left pad) in both halves
            nc.sync.dma_start(vpv[0:64, 0:1, 0:_HPW], c_zeros[0:64, 0:_HPW])
            nc.sync.dma_start(vpv[0:128, 0:117, 0:1], c_zeros[0:128, 0:117])
            fx = fxpool.tile([2, 7376], f32r, tag="fx", name="fx")
            fxv0 = fx[0:1, 1:7372].rearrange("p (r c) -> p r c", c=117)
            fxv1 = fx[1:2, 1:7372].rearrange("p (r c) -> p r c", c=117)
            nc.scalar.dma_start(fx[:], d_zflat)
            g = gsb[:, 112 * img:112 * img + 112]
            nc.scalar.dma_start(fxv0[0:1, 0:58, 0:112], g[0:58, :])
            nc.scalar.dma_start(fxv1[0:1, 0:58, 0:112], g[54:112, :])
            for si, (r0, nrows, fp) in enumerate(SEGS):
                seg = segts[(img * len(SEGS) + si) % 2]
                base = 1 - fp * (54 * 117)  # flat offset of X row 0 col 0 in this partition
                for dh in range(3):
                    for dw in range(3):
                        part = 3 * dh + dw
                        if r0 == 0 and dh == 0:
                            nc.sync.dma_start(seg[part:part + 1, 0:_SW],
                                              c_zeros[part:part + 1, 0:_SW])
                            nc.sync.dma_start(
                                seg[part:part + 1, _SW:nrows * _SW],
                                fx[fp:fp + 1, base + dw - 1:
                                   base + dw - 1 + (nrows - 1) * _SW])
                        else:
                            o = base + (r0 + dh - 1) * _SW + dw - 1
                            nc.sync.dma_start(seg[part:part + 1, 0:nrows * _SW],
                                              fx[fp:fp + 1, o:o + nrows * _SW])
                done = 0
                while done < nrows:
                    cr = min(4, nrows - done)
                    r = r0 + done
                    ps1 = c1psp.tile([64, cr * _SW], f32, tag="c1ps", name="c1ps")
                    rhs = seg[0:112, done * _SW:(done + cr) * _SW]
                    nc.tensor.matmul(ps1[:], c_w1im[:], rhs, start=True, stop=True)
                    ps1v = ps1[:].rearrange("p (r c) -> p r c", c=_SW)
                    # lo half: Hp rows r+1..r+cr (ACT relu+bias), skip 2 junk cols
                    nc.scalar.activation(vpv[0:64, r + 1:r + 1 + cr, 1:_HPW],
                                         ps1v[:, 0:cr, 0:115], AF.Relu,
                                         bias=c_b1rep[0:64, :])
                    # hi half (partition-shifted write): Hp+116 -> hi rows r.. (DVE)
                    nc.vector.tensor_scalar(vpv[64:128, r:r + cr, 1:_HPW],
                                            ps1v[:, 0:cr, 0:115], c_b1rep[0:64, :],
                                            0.0, op0=ALU.add, op1=ALU.max)
                    done += cr

        CH2 = [(r2, 4) for r2 in range(0, 108, 4)] + [(108, 3), (111, 3)]

        def conv2(img):
            vp = vps[img]
            vpv = vp[:].rearrange("p (r c) -> p r c", c=_HPW)
            for ci, (r2, cr) in enumerate(CH2):
                ps2 = c2psp.tile([128, cr * _VW], f32, tag="c2ps", name="c2ps")
                for w in range(3):
                    nc.tensor.matmul(ps2[:], c_w2p[w][:],
                                     vpv[0:128, r2:r2 + cr, w:w + _VW],
                                     start=(w == 0), stop=False)
                for w in range(3):
                    nc.tensor.matmul(ps2[:], c_w2s[w][:],
                                     vpv[0:128, r2 + 2:r2 + 2 + cr, w:w + _VW],
                                     start=False, stop=(w == 2))
                scr = scpool.tile([128, 4 * _VW], f32, tag="scr", name="scr")
                nc.scalar.activation(scr[:, 0:cr * _VW], ps2[:], AF.Relu,
                                     bias=c_b2t[:],
                                     accum_out=stats[:, 32 * img + ci:32 * img + ci + 1])

        conv1(0)
        conv1(1)
        conv2(0)
        conv1(2)
        conv2(1)
        conv1(3)
        conv2(2)
        conv2(3)

        with nc.allow_low_precision(reason="float32r holds identical fp32 bits"):
            for img in range(_IPC):
                nc.vector.reduce_sum(means[:, img:img + 1],
                                     stats[:, 32 * img:32 * img + 29], axis=AX.X)
        psf = c2psp.tile([_IPC, 256], f32, tag="psf", name="psf", bufs=1)
        nc.tensor.matmul(psf[:], means[:], c_wps[:], start=True, stop=False)
        nc.tensor.matmul(psf[:], c_ones1[:], c_brow[:], start=False, stop=True)
        outs = misc.tile([_IPC, 256], f32, tag="outs", name="outs")
        nc.scalar.copy(outs[:], psf[:])
        nc.sync.dma_start(out_ap[:], outs[:])

    nc.compile()
    return nc


_last_results = None


def kernel(x, w1, b1, g1, be1, m1, v1, w2, b2, g2, be2, m2, v2, wp, bp):
    import sys
    if '/opt/trn_rl_repo' not in sys.path:
        sys.path.insert(0, '/opt/trn_rl_repo')
    from concourse.bass_utils import run_bass_kernel_spmd

    global _last_results
    if 'nc' not in _PROG:
        _PROG['nc'] = _build_program()
    nc = _PROG['nc']

    consts = _prep_consts(w1, b1, g1, be1, m1, v1, w2, b2, g2, be2, m2, v2, wp, bp)
    x = np.ascontiguousarray(np.asarray(x, np.float32).reshape(_B_TOTAL, _N, _N))
    in_maps = []
    for c in range(_NCORES):
        m = dict(consts)
        m['x'] = x[c * _IPC:(c + 1) * _IPC]
        in_maps.append(m)
    res = run_bass_kernel_spmd(nc, in_maps, core_ids=list(range(_NCORES)))
    _last_results = res
    out = np.concatenate([res.results[c]['out'] for c in range(_NCORES)], axis=0)
    return out.astype(np.float32)


_B_TOTAL = 32
